# revision 32
# baseline (speedup 1.0000x reference)
"""Multi-head attention (B=4, N=2048, C=768, H=12, Dh=64) on 8 TRN2 NeuronCores.

Sharding (v4): head-parallel within each batch. Core (b, g) (g = core % 2)
computes q/k/v for heads 6g..6g+5 of batch b over the FULL 2048-token
sequence -- no duplicated K/V work between the pair -- runs attention for its
6 heads x 2048 queries, and produces a PARTIAL projection over its 384
channels. The host sums the two partial [2048, 768] outputs per batch
(host-side all-reduce; no device collectives). The bias rides in core g=0's
input; core g=1 receives zeros, keeping the program SPMD-identical.

Per-core inputs (partition dim first):
  xT     [768, 2048]  bf16  x[b].T (same for both cores of a pair)
  wqkvT  [768, 1152]  bf16  columns [q_g | k_g | v_g], 384 each, pre-sliced
  wprojT [384, 768]   bf16  proj_w.T rows for this core's 384 channels
  bias   [1, 768]     f32   real bias for g=0, zeros for g=1
  out    [2048, 768]  f32   partial projection

Pipeline (from trace analysis of the 371us baseline / 342us v2 / 335us v3):
  - scores for the two heads of a pair go to a [128, 1024] PSUM tile
    (2 banks), double-buffered; one 1024-wide exp per kv chunk on ScalarE.
    ScalarE (exp) is the saturated engine in steady state (~209us).
  - PV pair runs one step behind the scores/exp so the next exp is never
    blocked behind a PV waiting on the current one.
  - PV stationary is a 128-col window into the packed V tile
    [v_h|ones|v_{h+1}...] -> psum row 64 is the softmax denominator, rows
    65..127 garbage.
  - qkv/proj matmul chains keep 2 psum tiles resident so each weight load is
    reused by two matmuls.
  - q/k for head-pair 0 plus V chunk 0 run up front; V chunks 1-15 + q/k for
    pair 1 fill block (0,0); q/k pair 2 fills (0,1); per-ib projection fills
    the next ib's first block -- all interleaved into exp-bound steps.
  - normalize copies the pv psum body to SBUF immediately so the next block's
    PV accumulation is not blocked behind the reciprocal/broadcast chain.
"""

import sys

if "/opt/trn_rl_repo" not in sys.path:
    sys.path.insert(0, "/opt/trn_rl_repo")

import numpy as np
import ml_dtypes

B, N, C = 4, 2048, 768
H, Dh = 12, 64
HL = 6             # heads per core
CL = HL * Dh       # 384 local channels
SCALE = Dh ** -0.5
CCH = C // 128     # 6 contraction chunks (x/qkv input dim)
CCL = CL // 128    # 3 local head-pair chunks
NCORES = 8
VW = (HL - 1) * 65 + 128  # padded width of packed v tiles (453)

_NC_CACHE = {}


def _build():
    import concourse.bass as bass
    import concourse.tile as tile
    import concourse.mybir as mybir
    from concourse import bacc

    f32 = mybir.dt.float32
    bf16 = mybir.dt.bfloat16
    Exp = mybir.ActivationFunctionType.Exp

    nc = bacc.Bacc(
        "TRN2",
        target_bir_lowering=False,
        debug=False,
        enable_asserts=False,
        num_devices=NCORES,
    )

    xT = nc.dram_tensor("xT", [C, N], bf16, kind="ExternalInput").ap()
    wqkvT = nc.dram_tensor("wqkvT", [C, 3 * CL], bf16, kind="ExternalInput").ap()
    wprojT = nc.dram_tensor("wprojT", [CL, C], bf16, kind="ExternalInput").ap()
    bias = nc.dram_tensor("bias", [1, C], f32, kind="ExternalInput").ap()
    out = nc.dram_tensor("out", [N, C], f32, kind="ExternalOutput").ap()

    with tile.TileContext(nc) as tc:
        from contextlib import ExitStack

        with ExitStack() as ctx:
            singles = ctx.enter_context(tc.tile_pool(name="singles", bufs=1))
            psum = ctx.enter_context(tc.tile_pool(name="psum", bufs=1, space="PSUM"))
            work = ctx.enter_context(tc.tile_pool(name="work", bufs=4))

            # ---- input DMAs --------------------------------------------
            load = tc.alloc_tile_pool(name="load", bufs=1)
            xt = [load.tile([128, N], bf16, tag=f"xt{i}", name=f"xt{i}")
                  for i in range(CCH)]
            wq = [load.tile([128, 3 * CL], bf16, tag=f"wq{i}", name=f"wq{i}")
                  for i in range(CCH)]
            # xt on the sync queue; wq slices on the gpsimd queue ordered so
            # the first matmuls can start早: q/k pair-0 cols, v cols, rest.
            # everything on the fast HWDGE (sync) queue: SWDGE descriptor
            # generation costs ~700ns each on GpSimd and starts late
            for i in range(CCH):
                nc.sync.dma_start(out=xt[i][:, 0:512],
                                  in_=xT[i * 128:(i + 1) * 128, 0:512])
                nc.sync.dma_start(out=wq[i],
                                  in_=wqkvT[i * 128:(i + 1) * 128, :])
            for i in range(CCH):
                nc.sync.dma_start(out=xt[i][:, 512:N],
                                  in_=xT[i * 128:(i + 1) * 128, 512:N])
            wp = []
            for i in range(CCL):
                t = singles.tile([128, C], bf16, tag=f"wp{i}", name=f"wp{i}")
                nc.sync.dma_start(out=t, in_=wprojT[i * 128:(i + 1) * 128, :])
                wp.append(t)
            bias_bc = singles.tile([128, C], f32, tag="bias", name="bias_bc")
            nc.sync.dma_start(
                out=bias_bc,
                in_=bass.AP(tensor=bias.tensor, offset=bias.offset,
                            ap=[[0, 128]] + list(bias.ap[1:])),
            )

            # ---- qkv storage -------------------------------------------
            qt = [singles.tile([128, N], bf16, tag=f"qt{i}", name=f"qt{i}")
                  for i in range(CCL)]
            kt = [singles.tile([128, N], bf16, tag=f"kt{i}", name=f"kt{i}")
                  for i in range(CCL)]
            vt = [singles.tile([128, VW], bf16, tag=f"vt{i}", name=f"vt{i}")
                  for i in range(N // 128)]
            att = [singles.tile([128, N], bf16, tag=f"att{i}", name=f"att{i}")
                   for i in range(CCL)]

            # one q-or-k chain: 6 accumulating matmuls + a copy-out
            def emit_chain(dst, base, hp, nch):
                ops = []
                ps = psum.tile([128, 512], f32, tag="qk", bufs=2,
                               name=f"ch{base}{hp}{nch}")
                for cc in range(CCH):
                    ops.append(lambda base=base, hp=hp, nch=nch, cc=cc, ps=ps: nc.tensor.matmul(
                        ps,
                        lhsT=wq[cc][:, base + hp * 128:base + (hp + 1) * 128],
                        rhs=xt[cc][:, nch * 512:(nch + 1) * 512],
                        start=(cc == 0), stop=(cc == CCH - 1),
                    ))
                ops.append(lambda dst=dst, hp=hp, nch=nch, ps=ps: nc.vector.tensor_copy(
                    dst[hp][:, nch * 512:(nch + 1) * 512], ps))
                return ops

            def q_ch(hp, n):
                return emit_chain(qt, 0, hp, n)

            def k_ch(hp, n):
                return emit_chain(kt, CL, hp, n)

            # partial proj for one 128-row output block (6 matmuls + add + dma)
            def emit_proj(ic):
                ops = []
                pjs = [(psum.tile([128, 512], f32, tag="qk", bufs=2,
                                  name=f"pj{ic}_{d0}"), d0, dw)
                       for (d0, dw) in ((0, 512), (512, 256))]
                for cc in range(CCL):
                    for (pj, d0, dw) in pjs:
                        ops.append(lambda ic=ic, d0=d0, dw=dw, cc=cc, pj=pj: nc.tensor.matmul(
                            pj[:, 0:dw],
                            lhsT=att[cc][:, ic * 128:(ic + 1) * 128],
                            rhs=wp[cc][:, d0:d0 + dw],
                            start=(cc == 0), stop=(cc == CCL - 1),
                        ))
                def fin(ic=ic, pjs=pjs):
                    osb = work.tile([128, C], f32, tag="osb", bufs=3,
                                    name=f"osb{ic}")
                    for (pj, d0, dw) in pjs:
                        nc.vector.tensor_add(osb[:, d0:d0 + dw], pj[:, 0:dw],
                                             bias_bc[:, d0:d0 + dw])
                    nc.sync.dma_start(out=out[ic * 128:(ic + 1) * 128, :],
                                      in_=osb)
                ops.append(fin)
                return ops

            # v in [token, d] layout, packed [v_h(64)|1] x 6 heads + pad.
            def emit_vt(nt):
                ops = []
                vaug = vt[nt][:, 0:HL * 65].rearrange("p (h e) -> p h e", e=65)
                ops.append(lambda vaug=vaug: nc.vector.memset(
                    vaug[:, :, 64:65], 1.0))
                ops.append(lambda nt=nt: nc.vector.memset(
                    vt[nt][:, HL * 65:VW], 0.0))
                ps = psum.tile([128, 512], f32, tag="qk", bufs=2,
                               name=f"psv{nt}")
                for cc in range(CCH):
                    ops.append(lambda nt=nt, cc=cc, ps=ps: nc.tensor.matmul(
                        ps[:, 0:CL],
                        lhsT=xt[cc][:, nt * 128:(nt + 1) * 128],
                        rhs=wq[cc][:, 2 * CL:3 * CL],
                        start=(cc == 0), stop=(cc == CCH - 1),
                    ))
                ops.append(lambda vaug=vaug, ps=ps: nc.vector.tensor_copy(
                    vaug[:, :, 0:64],
                    ps[:, 0:CL].rearrange("p (h e) -> p h e", e=64),
                ))
                return ops

            # ---- phase 0: minimal prefix -------------------------------
            for op in q_ch(0, 0):
                op()
            for op in k_ch(0, 0):
                op()
            for op in emit_vt(0):
                op()

            # ---- attention ---------------------------------------------
            # per-block filler: matmul-ish ops interleaved into exp-bound
            # steps so the PE never idles while ScalarE runs exp.
            NJ = N // 128                     # 16 kv chunks
            filler = []
            # pair-major block order: each head pair's q/k chains are
            # produced inside the PREVIOUS pair's exp-bound blocks.
            fill_plan = {
                (1, 0): lambda: q_ch(0, 2) + q_ch(1, 0) + k_ch(1, 0),
                (2, 0): lambda: q_ch(0, 3) + k_ch(1, 1) + k_ch(1, 2),
                (3, 0): lambda: k_ch(1, 3) + q_ch(1, 1),
                (0, 1): lambda: q_ch(1, 2) + q_ch(2, 0) + k_ch(2, 0),
                (1, 1): lambda: q_ch(1, 3) + k_ch(2, 1),
                (2, 1): lambda: k_ch(2, 2) + k_ch(2, 3),
                (3, 1): lambda: q_ch(2, 1),
                (0, 2): lambda: q_ch(2, 2),
                (1, 2): lambda: (q_ch(2, 3)
                                 + [op for ic in range(0, 4)
                                    for op in emit_proj(ic)]),
                (2, 2): lambda: [op for ic in range(4, 8)
                                 for op in emit_proj(ic)],
                (3, 2): lambda: [op for ic in range(8, 12)
                                 for op in emit_proj(ic)],
            }
            per_steps = {(0, 0): 11, (1, 0): 2, (2, 0): 2, (3, 0): 1,
                         (0, 1): 2, (1, 1): 1, (2, 1): 1, (3, 1): 1,
                         (0, 2): 1, (1, 2): 3, (2, 2): 2, (3, 2): 2}
            for hp in range(CCL):             # local head pair (outer!)
                for ib in range(N // 512):    # 512-wide query block
                    if (ib, hp) == (2, 2):
                        load.release()
                    if (ib, hp) == (0, 0):
                        # weave: vt chunks stay ahead of their PV; k pair-0
                        # chunks m land before this block's step 4m; q0 chunk
                        # 1 is consumed by the very next block.
                        filler = []
                        vts = [emit_vt(nt) for nt in range(1, NJ)]
                        filler += vts[0] + k_ch(0, 1) + vts[1] + vts[2]
                        filler += k_ch(0, 2) + vts[3] + vts[4] + k_ch(0, 3)
                        for v in vts[5:]:
                            filler += v
                        filler += q_ch(0, 1)
                    else:
                        filler = fill_plan.get((ib, hp), lambda: [])()
                    if (ib, hp) == (0, 0):
                        ramp = [3, 4, 5, 6, 8, 10, 14, 16,
                                16, 16, 16, 16, 16, 16, 16, 16]
                    else:
                        ramp = [per_steps[(ib, hp)]] * NJ
                    pv = [psum.tile([128, 512], f32, tag="pv", bufs=2,
                                    name=f"pv{h2}") for h2 in range(2)]
                    pv_q = []
                    for j in range(NJ):       # one kv chunk per step
                        st = psum.tile([128, 1024], f32, tag="st", bufs=2,
                                       name="st")
                        for h2 in range(2):
                            hb = h2 * 64
                            nc.tensor.matmul(
                                st[:, h2 * 512:(h2 + 1) * 512],
                                lhsT=kt[hp][hb:hb + 64, j * 128:(j + 1) * 128],
                                rhs=qt[hp][hb:hb + 64, ib * 512:(ib + 1) * 512],
                                start=True, stop=True,
                            )
                        et = work.tile([128, 1024], bf16, tag="et", bufs=4,
                                       name="et")
                        nc.scalar.activation(et, st, Exp, scale=SCALE)
                        for _ in range(ramp[j]):
                            if filler:
                                filler.pop(0)()
                        if len(pv_q) >= 2:
                            pv_q.pop(0)()

                        def mk_pv(j=j, et=et, pv=pv, hp=hp):
                            for h2 in range(2):
                                h = hp * 2 + h2
                                nc.tensor.matmul(
                                    pv[h2],
                                    lhsT=vt[j][:, h * 65:h * 65 + 128],
                                    rhs=et[:, h2 * 512:(h2 + 1) * 512],
                                    start=(j == 0), stop=(j == NJ - 1),
                                )
                        pv_q.append(mk_pv)
                    # force-drain BEFORE the last PVs: leftover matmuls fill
                    # the PE bubble while the final exps of the block run
                    while filler:
                        filler.pop(0)()
                    for f in pv_q:
                        f()
                    srow = work.tile([1, 1024], f32, tag="srow", bufs=2,
                                     name="srow")
                    pvbs = []
                    for h2 in range(2):
                        nc.vector.tensor_copy(srow[0:1, h2 * 512:(h2 + 1) * 512],
                                              pv[h2][64:65, :])
                        pvb = work.tile([64, 512], f32, tag="pvb", bufs=4,
                                        name="pvb")
                        nc.vector.tensor_copy(pvb, pv[h2][0:64, :])
                        pvbs.append(pvb)
                    sinv = work.tile([1, 1024], f32, tag="sinv", bufs=2,
                                     name="sinv")
                    nc.vector.reciprocal_approx_fast(sinv, srow)
                    for h2 in range(2):
                        bc = work.tile([64, 512], f32, tag="bc", bufs=4,
                                       name="bc")
                        nc.gpsimd.partition_broadcast(
                            bc, sinv[0:1, h2 * 512:(h2 + 1) * 512])
                        nc.vector.tensor_mul(
                            att[hp][h2 * 64:h2 * 64 + 64, ib * 512:(ib + 1) * 512],
                            pvbs[h2],
                            bc,
                        )
            while filler:
                filler.pop(0)()
            # tail: proj for the last query block
            for ic in range(12, 16):
                for op in emit_proj(ic):
                    op()

    nc.compile()
    return nc


def _get_nc():
    if "nc" not in _NC_CACHE:
        _NC_CACHE["nc"] = _build()
    return _NC_CACHE["nc"]


def _ensure_ntff_hook():
    """The agent image's ``antenv`` lacks ``axon_hooks``; synthesize it so
    ``run_bass_kernel_spmd(trace=True)`` can capture NTFF profiles."""
    import types
    try:
        from antenv.axon_hooks import get_axon_ntff_profile_hook  # noqa: F401
        return
    except ImportError:
        pass
    import antenv
    from trn_agent_boot.trn_boot import _ntff_profile_via_ctypes
    hook = _ntff_profile_via_ctypes("/opt/axon/libaxon_pjrt.so")
    mod = types.ModuleType("antenv.axon_hooks")
    mod._hook = hook
    mod.get_axon_ntff_profile_hook = lambda: mod._hook

    def _set(h):
        mod._hook = h

    mod.set_axon_ntff_profile_hook = _set
    sys.modules["antenv.axon_hooks"] = mod
    antenv.axon_hooks = mod


def kernel(trace=False, **inputs):
    x = np.asarray(inputs["x"], np.float32)
    qkv_w = np.asarray(inputs["qkv_w"], np.float32)
    proj_w = np.asarray(inputs["proj_w"], np.float32)
    proj_b = np.asarray(inputs["proj_b"], np.float32)

    nc = _get_nc()

    xTb = np.ascontiguousarray(x.transpose(0, 2, 1)).astype(ml_dtypes.bfloat16)
    wqkvT = np.ascontiguousarray(qkv_w.T).astype(ml_dtypes.bfloat16)  # [768, 2304]
    wprojT = np.ascontiguousarray(proj_w.T).astype(ml_dtypes.bfloat16)  # [768, 768]
    bias = np.ascontiguousarray(proj_b.reshape(1, C)).astype(np.float32)
    zbias = np.zeros_like(bias)

    in_maps = []
    for c in range(NCORES):
        b, g = divmod(c, 2)
        cols = slice(g * CL, (g + 1) * CL)
        wq_loc = np.concatenate(
            [wqkvT[:, 0:C][:, cols], wqkvT[:, C:2 * C][:, cols],
             wqkvT[:, 2 * C:3 * C][:, cols]], axis=1)
        in_maps.append({
            "xT": xTb[b],
            "wqkvT": np.ascontiguousarray(wq_loc),
            "wprojT": np.ascontiguousarray(wprojT[g * CL:(g + 1) * CL, :]),
            "bias": bias if g == 0 else zbias,
        })

    from concourse import bass_utils
    if trace:
        _ensure_ntff_hook()
        bass_utils.upload_artifacts = lambda tmpdir: tmpdir
    res = bass_utils.run_bass_kernel_spmd(
        nc, in_maps, core_ids=list(range(NCORES)), trace=trace,
    )

    out = np.empty((B, N, C), np.float32)
    for b in range(B):
        out[b] = res.results[2 * b]["out"]
        out[b] += res.results[2 * b + 1]["out"]

    if trace:
        return out, res
    return out
